# revision 1
# baseline (speedup 1.0000x reference)
"""Trainium2 Bass kernel for nn_IterativeStructureRefiner.

Math restructuring (validated vs reference to ~1e-7):
  Per iteration (s = structure, cs = continuity*s):
    num = oxx.(hx + dpq/2) + oyy.(vy + dpq/2) + oxy.dmq
      where hx = csL+csR, vy/dpq/dmq are banded vertical stencils:
      vy = T0@cs, dpq = T0@hx, dmq = A0@(csL-csR)   (T0 = super+sub diag,
      A0 = super-sub diag) -> computed on the TensorEngine into PSUM:
        Sxx = (I + T0/2)@hx,   Syy = T0@cs + (T0/2)@hx,   dmq = A0@hd
    sm  = T9@(sL+sC+sR)      (T9 = tridiag(1/9): 3x3 box mean)
    s'  = 0.75*s + 0.25*g . (sm - s + r.num)
      with g = clip(1-clip(unc,0,1),0,1), r = 1/(den+1e-6) precomputed,
      den = same num-structure applied to continuity. The reference's final
      clip(.,0,1) is provably inactive (pre-clip in [0.5s, 0.5+0.5s]).

Sharding: pure data-parallel, one batch image per NeuronCore (B=8, 8 cores).
Each image is processed as 9x2 patches of [128 rows x 524 cols] with a 6-px
halo (interior 116x512): all 6 iterations run locally per patch -> zero
cross-patch/iteration HBM traffic; inputs are read once, output written once.
"""

import numpy as np

H = W = 1024
PATCH_W = 524          # patch-col space: image cols [c0-6, c0+518)
TILE_W = PATCH_W + 2   # +1 zero-pad col each side for shifted reads
ROWS_OUT = 116         # 128 partitions - 2*6 halo
NUM_ITERS = 6
EPS = 1e-6

_CACHE = {}


def _build_bass():
    import concourse.bacc as bacc
    import concourse.mybir as mybir
    from concourse.tile import TileContext

    fp32 = mybir.dt.float32
    Alu = mybir.AluOpType
    Act = mybir.ActivationFunctionType

    # Bacc (not raw Bass): its compile pass legalizes multi-sem waits, which
    # walrus codegen rejects ("Too many sync wait commands").
    nc = bacc.Bacc("TRN2", debug=False)

    cen_d = nc.dram_tensor("center", [H, W], fp32, kind="ExternalInput")
    con_d = nc.dram_tensor("continuity", [H, W], fp32, kind="ExternalInput")
    ori_d = nc.dram_tensor("orientation", [2, H, W], fp32, kind="ExternalInput")
    unc_d = nc.dram_tensor("uncertainty", [H, W], fp32, kind="ExternalInput")
    out_d = nc.dram_tensor("out", [H, W], fp32, kind="ExternalOutput")

    # Stationary matrices for the banded vertical stencils. matmul computes
    # lhsT.T @ rhs with contraction over partitions: out[m,:] = sum_k St[k,m]*in[k,:]
    k = np.arange(128)
    T0 = ((np.abs(k[:, None] - k[None, :]) == 1)).astype(np.float32)       # in[m-1]+in[m+1]
    Bm = np.eye(128, dtype=np.float32) + 0.5 * T0                          # in[m] + .5*(in[m-1]+in[m+1])
    H0 = 0.5 * T0
    A0 = ((k[:, None] == k[None, :] - 1).astype(np.float32)
          - (k[:, None] == k[None, :] + 1).astype(np.float32))             # in[m-1]-in[m+1]
    T9 = ((np.abs(k[:, None] - k[None, :]) <= 1)).astype(np.float32) / 9.0

    st_drams = [nc.inline_tensor(m, name=f"st_{i}")
                for i, m in enumerate([T0, Bm, H0, A0, T9])]
    # bottom row-panel valid-partition mask (p < 102): compute-engine APs
    # can't start at partition 102, so apply as per-partition tensor_scalar
    botmask_np = (np.arange(128) < 102).astype(np.float32)[:, None]
    bot_dram = nc.inline_tensor(botmask_np, name="botmask")

    row_panels = []
    for r0 in range(0, H, ROWS_OUT):
        r1 = min(r0 + ROWS_OUT, H)
        row_panels.append((r0, r1))
    col_panels = [0, 512]

    with TileContext(nc) as tc:
        with (
            tc.tile_pool(name="consts", bufs=1) as cpool,
            tc.tile_pool(name="inp", bufs=3) as ipool,
            tc.tile_pool(name="pre", bufs=2) as ppool,
            tc.tile_pool(name="scr", bufs=2) as spool,
            tc.tile_pool(name="psum", bufs=1, space="PSUM") as qpool,
        ):
            # stationaries -> SBUF once
            st = []
            for i, d in enumerate(st_drams):
                t = cpool.tile([128, 128], fp32, tag=f"st{i}")
                nc.sync.dma_start(out=t[:], in_=d[:, :])
                st.append(t)
            tT0, tB, tH0, tA0, tT9 = st
            botmask = cpool.tile([128, 1], fp32, tag="botmask")
            nc.sync.dma_start(out=botmask[:], in_=bot_dram[:, :])

            # persistent ping-pong structure tiles (edge cols zeroed once;
            # iteration writes cover [1:TILE_W-1] only)
            s_ab = []
            for nm in ("s_a", "s_b"):
                t = cpool.tile([128, TILE_W], fp32, tag=nm)
                nc.vector.memset(t[:, 0:1], 0.0)
                nc.vector.memset(t[:, TILE_W - 1:TILE_W], 0.0)
                s_ab.append(t)
            # persistent cs tile, same edge discipline
            cs = cpool.tile([128, TILE_W], fp32, tag="cs")
            nc.vector.memset(cs[:, 0:1], 0.0)
            nc.vector.memset(cs[:, TILE_W - 1:TILE_W], 0.0)

            for (r0, r1) in row_panels:
                for c0 in col_panels:
                    # ---- load inputs with halo; tile col t <-> image col c0-7+t
                    img_lo = max(c0 - 7, 0)
                    img_hi = min(c0 + 519, W)
                    t_lo = img_lo - (c0 - 7)
                    t_hi = img_hi - (c0 - 7)
                    row_lo = max(r0 - 6, 0)
                    row_hi = min(r0 + 122, H)
                    p_lo = row_lo - (r0 - 6)
                    p_hi = row_hi - (r0 - 6)

                    def load(src_ap, tag):
                        t = ipool.tile([128, TILE_W], fp32, tag=tag)
                        if t_lo > 0:
                            nc.gpsimd.memset(t[:, 0:t_lo], 0.0)
                        if t_hi < TILE_W:
                            nc.gpsimd.memset(t[:, t_hi:TILE_W], 0.0)
                        if p_lo > 0:
                            nc.gpsimd.memset(t[0:p_lo, t_lo:t_hi], 0.0)
                        if p_hi < 128:
                            # compute-engine APs need 32-aligned partition start;
                            # DMA below overwrites [aligned_lo:p_hi)
                            aligned_lo = (p_hi // 32) * 32
                            nc.gpsimd.memset(t[aligned_lo:128, t_lo:t_hi], 0.0)
                        nc.sync.dma_start(
                            out=t[p_lo:p_hi, t_lo:t_hi],
                            in_=src_ap[row_lo:row_hi, img_lo:img_hi])
                        return t

                    s0 = load(cen_d, "s0")
                    cont = load(con_d, "cont")
                    ox = load(ori_d[0], "ox")
                    oy = load(ori_d[1], "oy")
                    unc = load(unc_d, "unc")

                    # ---- per-patch precompute ----
                    oxx = ppool.tile([128, TILE_W], fp32, tag="oxx")
                    oyy = ppool.tile([128, TILE_W], fp32, tag="oyy")
                    oxy = ppool.tile([128, TILE_W], fp32, tag="oxy")
                    g4 = ppool.tile([128, TILE_W], fp32, tag="g4")
                    rmap = ppool.tile([128, PATCH_W], fp32, tag="rmap")
                    nc.scalar.activation(oxx[:], ox[:], Act.Square)
                    nc.scalar.activation(oyy[:], oy[:], Act.Square)
                    nc.gpsimd.tensor_mul(out=oxy[:], in0=ox[:], in1=oy[:])
                    c1 = spool.tile([128, TILE_W], fp32, tag="c1")
                    nc.vector.tensor_scalar(
                        out=c1[:], in0=unc[:], scalar1=1.0, scalar2=0.0,
                        op0=Alu.min, op1=Alu.max)
                    nc.vector.tensor_scalar(
                        out=g4[:], in0=c1[:], scalar1=-0.25, scalar2=0.25,
                        op0=Alu.mult, op1=Alu.add)
                    # Zero g4 on out-of-image pad regions: the update then
                    # leaves s=0 there every iteration, reproducing the
                    # reference's per-iteration zero padding at image edges.
                    if t_lo > 0:
                        nc.vector.memset(g4[:, 0:t_lo], 0.0)
                    if t_hi < TILE_W:
                        nc.vector.memset(g4[:, t_hi:TILE_W], 0.0)
                    if p_lo > 0:
                        nc.vector.memset(g4[0:p_lo, :], 0.0)
                    if p_hi < 128:
                        assert p_hi == 102
                        nc.vector.tensor_scalar(
                            out=g4[:], in0=g4[:], scalar1=botmask[:, 0:1],
                            scalar2=None, op0=Alu.mult)

                    IN = slice(1, 1 + PATCH_W)   # tile cols holding patch-col space

                    def vstencils(src_tile, hx_t, hd_t, sxx_q, syy_q, dmq_q):
                        """hx/hd from src (526-wide), then PE stencils into PSUM."""
                        nc.vector.tensor_add(out=hx_t[:], in0=src_tile[:, 0:PATCH_W],
                                             in1=src_tile[:, 2:TILE_W])
                        nc.gpsimd.tensor_sub(out=hd_t[:], in0=src_tile[:, 0:PATCH_W],
                                             in1=src_tile[:, 2:TILE_W])
                        for lo in (0, 512):
                            hi = min(lo + 512, PATCH_W)
                            nc.tensor.matmul(sxx_q[:, lo:hi], tB[:], hx_t[:, lo:hi],
                                             start=True, stop=True)
                            nc.tensor.matmul(syy_q[:, lo:hi], tT0[:],
                                             src_tile[:, 1 + lo:1 + hi], start=True, stop=False)
                            nc.tensor.matmul(syy_q[:, lo:hi], tH0[:], hx_t[:, lo:hi],
                                             start=False, stop=True)
                            nc.tensor.matmul(dmq_q[:, lo:hi], tA0[:], hd_t[:, lo:hi],
                                             start=True, stop=True)

                    def weighted_num(sxx_q, syy_q, dmq_q, out_t, tmp1, tmp2, tmp3):
                        nc.vector.tensor_mul(out=tmp1[:], in0=oxx[:, IN], in1=sxx_q[:])
                        nc.vector.tensor_mul(out=tmp2[:], in0=oyy[:, IN], in1=syy_q[:])
                        nc.vector.tensor_mul(out=tmp3[:], in0=oxy[:, IN], in1=dmq_q[:])
                        nc.vector.tensor_add(out=tmp1[:], in0=tmp1[:], in1=tmp2[:])
                        nc.gpsimd.tensor_add(out=out_t[:], in0=tmp1[:], in1=tmp3[:])

                    # den -> r
                    hxc = spool.tile([128, PATCH_W], fp32, tag="hx")
                    hdc = spool.tile([128, PATCH_W], fp32, tag="hd")
                    q_sxx = qpool.tile([128, PATCH_W], fp32, tag="q_sxx")
                    q_syy = qpool.tile([128, PATCH_W], fp32, tag="q_syy")
                    q_dmq = qpool.tile([128, PATCH_W], fp32, tag="q_dmq")
                    vstencils(cont, hxc, hdc, q_sxx, q_syy, q_dmq)
                    d1 = spool.tile([128, PATCH_W], fp32, tag="u1")
                    d2 = spool.tile([128, PATCH_W], fp32, tag="u2")
                    d3 = spool.tile([128, PATCH_W], fp32, tag="u3")
                    den = spool.tile([128, PATCH_W], fp32, tag="num")
                    weighted_num(q_sxx, q_syy, q_dmq, den, d1, d2, d3)
                    nc.vector.tensor_scalar_add(rmap[:], den[:], EPS)
                    nc.vector.reciprocal_approx_fast(out=rmap[:], in_=rmap[:])

                    # ---- 6 iterations ----
                    s_cur = s0
                    for it in range(NUM_ITERS):
                        s_nxt = s_ab[it % 2]
                        nc.gpsimd.tensor_mul(out=cs[:, IN], in0=cont[:, IN],
                                             in1=s_cur[:, IN])
                        hx = spool.tile([128, PATCH_W], fp32, tag="hx")
                        hd = spool.tile([128, PATCH_W], fp32, tag="hd")
                        q_sxx = qpool.tile([128, PATCH_W], fp32, tag="q_sxx")
                        q_syy = qpool.tile([128, PATCH_W], fp32, tag="q_syy")
                        q_dmq = qpool.tile([128, PATCH_W], fp32, tag="q_dmq")
                        vstencils(cs, hx, hd, q_sxx, q_syy, q_dmq)

                        # smooth: hs3 = sL+sC+sR ; sm = T9@hs3
                        hs3a = spool.tile([128, PATCH_W], fp32, tag="hs3a")
                        hs3 = spool.tile([128, PATCH_W], fp32, tag="hs3")
                        nc.vector.tensor_add(out=hs3a[:], in0=s_cur[:, 0:PATCH_W],
                                             in1=s_cur[:, 2:TILE_W])
                        nc.gpsimd.tensor_add(out=hs3[:], in0=hs3a[:],
                                             in1=s_cur[:, IN])
                        q_sm = qpool.tile([128, PATCH_W], fp32, tag="q_sm")
                        for lo in (0, 512):
                            hi = min(lo + 512, PATCH_W)
                            nc.tensor.matmul(q_sm[:, lo:hi], tT9[:], hs3[:, lo:hi],
                                             start=True, stop=True)

                        u1 = spool.tile([128, PATCH_W], fp32, tag="u1")
                        u2 = spool.tile([128, PATCH_W], fp32, tag="u2")
                        u3 = spool.tile([128, PATCH_W], fp32, tag="u3")
                        num = spool.tile([128, PATCH_W], fp32, tag="num")
                        weighted_num(q_sxx, q_syy, q_dmq, num, u1, u2, u3)

                        w1 = spool.tile([128, PATCH_W], fp32, tag="w1")
                        w2a = spool.tile([128, PATCH_W], fp32, tag="w2a")
                        w2 = spool.tile([128, PATCH_W], fp32, tag="w2")
                        uu = spool.tile([128, PATCH_W], fp32, tag="uu")
                        nc.vector.tensor_mul(out=w1[:], in0=rmap[:], in1=num[:])
                        nc.vector.tensor_sub(out=w2a[:], in0=q_sm[:], in1=s_cur[:, IN])
                        nc.gpsimd.tensor_add(out=w2[:], in0=w2a[:], in1=w1[:])
                        nc.vector.tensor_mul(out=uu[:], in0=g4[:, IN], in1=w2[:])
                        nc.vector.scalar_tensor_tensor(
                            out=s_nxt[:, IN], in0=s_cur[:, IN], scalar=0.75,
                            in1=uu[:], op0=Alu.mult, op1=Alu.add)
                        s_cur = s_nxt

                    # ---- store interior ----
                    nrows = r1 - r0
                    nc.sync.dma_start(
                        out=out_d[r0:r1, c0:c0 + 512],
                        in_=s_cur[6:6 + nrows, 7:519])

    nc.finalize()
    return nc


def kernel(center, continuity, orientation, uncertainty):
    from concourse.bass_utils import run_bass_kernel_spmd

    if "nc" not in _CACHE:
        _CACHE["nc"] = _build_bass()
    nc = _CACHE["nc"]

    B = center.shape[0]
    in_maps = []
    for b in range(B):
        in_maps.append({
            "center": np.ascontiguousarray(center[b, 0]),
            "continuity": np.ascontiguousarray(continuity[b, 0]),
            "orientation": np.ascontiguousarray(orientation[b]),
            "uncertainty": np.ascontiguousarray(uncertainty[b, 0]),
        })
    res = run_bass_kernel_spmd(nc, in_maps, core_ids=list(range(B)))
    out = np.stack([r["out"] for r in res.results])[:, None]
    return out.astype(np.float32)



# revision 2
# speedup vs baseline: 1.4952x; 1.4952x over previous
"""Trainium2 Bass kernel for nn_IterativeStructureRefiner (v2, bf16).

Math (validated vs reference: fp32 9e-8, bf16-quantized 3e-3 l2rel):
  Rank-3 orientation factorization of the 8-neighbor affinity:
    num = oxx*S1 + oyy*S2 + oxy*S3,  S1 = Bm@hx, S2 = T0@cs + H0@hx,
    S3 = A0@csL - A0@csR   (cs = continuity*s, hx = csL+csR; vertical
    stencils are banded 128x128 stationaries on the TensorEngine).
  den = same stencils on continuity (iteration-invariant) -> folded with
  the uncertainty gate into precomputed coefficient maps:
    Cxx = 0.25*g*r*oxx (etc.), G9 = 0.25*g/9, r = 1/(den+eps).
  Smooth term via one PE pass: Q9 = T0I@sL + (T0I-9I)@sC + T0I@sR
    = 9*(box3x3(s) - s), so  s' = 0.75*s + G9*Q9 + Cxx*S1+Cyy*S2+Cxy*S3.
  The reference's final clip is provably inactive.

All loop tensors are bf16 (DVE 2x packing; PE bf16 matmuls ~3x faster than
fp32; PSUM accumulates fp32). Column layout keeps every DVE operand at an
even (4B-aligned) offset so bf16 packing engages; odd-offset shifted reads
only ever feed the PE. PSUM->SBUF drains ride the otherwise-idle Scalar
engine; GpSimd takes two elementwise ops per iteration.

Sharding: pure data-parallel, one batch image per NeuronCore (B=8).
9x2 patches of [128 rows x 524 cols] with 6px halo (interior 116x512);
all 6 iterations run locally per patch -> inputs read once, output
written once.
"""

import numpy as np

H = W = 1024
TILE_W = 528           # tile col t <-> image col c0 - 8 + t ; cols 0,527 pad
W0, W1 = 2, 526        # working window (524 cols), psum col j <-> tile col j+2
PW = W1 - W0           # 524
INT0, INT1 = 8, 520    # interior output cols = image [c0, c0+512)
ROWS_OUT = 116         # 128 partitions - 2*6 halo
NUM_ITERS = 6
EPS = 1e-6

_CACHE = {}


def _build_bass():
    import concourse.bacc as bacc
    import concourse.mybir as mybir
    from concourse.tile import TileContext

    fp32 = mybir.dt.float32
    bf16 = mybir.dt.bfloat16
    Alu = mybir.AluOpType
    Act = mybir.ActivationFunctionType

    nc = bacc.Bacc("TRN2", debug=False)

    cen_d = nc.dram_tensor("center", [H, W], fp32, kind="ExternalInput")
    con_d = nc.dram_tensor("continuity", [H, W], fp32, kind="ExternalInput")
    ori_d = nc.dram_tensor("orientation", [2, H, W], fp32, kind="ExternalInput")
    unc_d = nc.dram_tensor("uncertainty", [H, W], fp32, kind="ExternalInput")
    out_d = nc.dram_tensor("out", [H, W], fp32, kind="ExternalOutput")

    # Banded stationaries; matmul computes lhsT.T @ rhs over partitions:
    # out[m,:] = sum_k St[k,m] * in[k,:]
    k = np.arange(128)
    T0 = (np.abs(k[:, None] - k[None, :]) == 1).astype(np.float32)  # in[m-1]+in[m+1]
    Bm = np.eye(128, dtype=np.float32) + 0.5 * T0
    H0 = 0.5 * T0
    A0 = ((k[:, None] == k[None, :] - 1).astype(np.float32)
          - (k[:, None] == k[None, :] + 1).astype(np.float32))      # in[m-1]-in[m+1]
    A0m = -A0
    T0I = np.eye(128, dtype=np.float32) + T0                         # 3-row sum
    T0I9 = T0 - 8.0 * np.eye(128, dtype=np.float32)                  # T0I - 9I
    ST = [Bm, T0, H0, A0, A0m, T0I, T0I9]
    st_drams = [nc.inline_tensor(m, name=f"st_{i}") for i, m in enumerate(ST)]
    botmask_np = (np.arange(128) < 102).astype(np.float32)[:, None]
    bot_dram = nc.inline_tensor(botmask_np, name="botmask")

    row_panels = []
    for r0 in range(0, H, ROWS_OUT):
        row_panels.append((r0, min(r0 + ROWS_OUT, H)))
    col_panels = [0, 512]

    with TileContext(nc) as tc:
        with (
            tc.tile_pool(name="consts", bufs=1) as cpool,
            tc.tile_pool(name="inp", bufs=2) as ipool,
            tc.tile_pool(name="pre", bufs=2) as ppool,
            tc.tile_pool(name="scr", bufs=2) as spool,
            tc.tile_pool(name="psum", bufs=1, space="PSUM") as qpool,
        ):
            # stationaries -> SBUF fp32 staging -> bf16 cast (once)
            st_b = []
            for i, d in enumerate(st_drams):
                tf = spool.tile([128, 128], fp32, tag=f"stf{i}")
                nc.sync.dma_start(out=tf[:], in_=d[:, :])
                tb = cpool.tile([128, 128], bf16, tag=f"st{i}")
                nc.scalar.copy(tb[:], tf[:])
                st_b.append(tb)
            tBm, tT0, tH0, tA0, tA0m, tT0I, tT0I9 = st_b
            botmask = cpool.tile([128, 1], fp32, tag="botmask")
            nc.sync.dma_start(out=botmask[:], in_=bot_dram[:, :])

            # persistent bf16 ping-pong structure tiles; edge cols (outside
            # the written window [W0:W1)) zeroed once
            s_ab = []
            for nm in ("s_a", "s_b"):
                t = cpool.tile([128, TILE_W], bf16, tag=nm)
                nc.vector.memset(t[:, 0:W0], 0.0)
                nc.vector.memset(t[:, W1:TILE_W], 0.0)
                s_ab.append(t)

            for (r0, r1) in row_panels:
                for c0 in col_panels:
                    # ---- input mapping: tile col t <-> image col c0-8+t
                    img_lo = max(c0 - 7, 0)
                    img_hi = min(c0 + 519, W)
                    t_lo = img_lo - (c0 - 8)
                    t_hi = img_hi - (c0 - 8)
                    row_lo = max(r0 - 6, 0)
                    row_hi = min(r0 + 122, H)
                    p_lo = row_lo - (r0 - 6)
                    p_hi = row_hi - (r0 - 6)

                    def load(src_ap, tag):
                        t = ipool.tile([128, TILE_W], fp32, tag=tag)
                        if t_lo > 0:
                            nc.gpsimd.memset(t[:, 0:t_lo], 0.0)
                        if t_hi < TILE_W:
                            nc.gpsimd.memset(t[:, t_hi:TILE_W], 0.0)
                        if p_lo > 0:
                            nc.gpsimd.memset(t[0:p_lo, t_lo:t_hi], 0.0)
                        if p_hi < 128:
                            aligned_lo = (p_hi // 32) * 32
                            nc.gpsimd.memset(t[aligned_lo:128, t_lo:t_hi], 0.0)
                        nc.sync.dma_start(
                            out=t[p_lo:p_hi, t_lo:t_hi],
                            in_=src_ap[row_lo:row_hi, img_lo:img_hi])
                        return t

                    t_cen = load(cen_d, "cen")
                    t_con = load(con_d, "con")
                    t_ox = load(ori_d[0], "ox")
                    t_oy = load(ori_d[1], "oy")
                    t_unc = load(unc_d, "unc")

                    WSL = slice(W0, W1)

                    # ---- per-patch precompute ----
                    cont_b = ppool.tile([128, TILE_W], bf16, tag="cont_b")
                    s0_b = ppool.tile([128, TILE_W], bf16, tag="s0_b")
                    nc.scalar.copy(cont_b[:], t_con[:])
                    nc.scalar.copy(s0_b[:], t_cen[:])
                    oxx = ppool.tile([128, PW], bf16, tag="oxx")
                    oyy = ppool.tile([128, PW], bf16, tag="oyy")
                    oxy = ppool.tile([128, PW], bf16, tag="oxy")
                    nc.scalar.activation(oxx[:], t_ox[:, WSL], Act.Square)
                    nc.scalar.activation(oyy[:], t_oy[:, WSL], Act.Square)
                    nc.gpsimd.tensor_mul(out=oxy[:], in0=t_ox[:, WSL],
                                         in1=t_oy[:, WSL])

                    # den stencils on continuity
                    hxC = spool.tile([128, 526], bf16, tag="hx")
                    nc.vector.tensor_add(out=hxC[:], in0=cont_b[:, 0:526],
                                         in1=cont_b[:, 2:528])
                    q1 = qpool.tile([128, PW], fp32, tag="q1")
                    q2 = qpool.tile([128, PW], fp32, tag="q2")
                    q3 = qpool.tile([128, PW], fp32, tag="q3")

                    def stencils(src, hxt, q1t, q2t, q3t):
                        """q1=Bm@hx, q2=T0@src+H0@hx, q3=A0@srcL-A0@srcR over
                        the working window; src is a [128,TILE_W] bf16 tile."""
                        for lo in (0, 512):
                            hi = min(lo + 512, PW)
                            nc.tensor.matmul(q1t[:, lo:hi], tBm[:],
                                             hxt[:, 1 + lo:1 + hi],
                                             start=True, stop=True)
                            nc.tensor.matmul(q2t[:, lo:hi], tT0[:],
                                             src[:, 2 + lo:2 + hi],
                                             start=True, stop=False)
                            nc.tensor.matmul(q2t[:, lo:hi], tH0[:],
                                             hxt[:, 1 + lo:1 + hi],
                                             start=False, stop=True)
                            nc.tensor.matmul(q3t[:, lo:hi], tA0[:],
                                             src[:, 1 + lo:1 + hi],
                                             start=True, stop=False)
                            nc.tensor.matmul(q3t[:, lo:hi], tA0m[:],
                                             src[:, 3 + lo:3 + hi],
                                             start=False, stop=True)

                    stencils(cont_b, hxC, q1, q2, q3)
                    p1 = spool.tile([128, PW], fp32, tag="p1")
                    p2 = spool.tile([128, PW], fp32, tag="p2")
                    p3 = spool.tile([128, PW], fp32, tag="p3")
                    nc.vector.tensor_mul(out=p1[:], in0=oxx[:], in1=q1[:])
                    nc.vector.tensor_mul(out=p2[:], in0=oyy[:], in1=q2[:])
                    nc.vector.tensor_mul(out=p3[:], in0=oxy[:], in1=q3[:])
                    a1 = spool.tile([128, PW], fp32, tag="a1")
                    den = spool.tile([128, PW], fp32, tag="den")
                    nc.gpsimd.tensor_add(out=a1[:], in0=p1[:], in1=p2[:])
                    nc.gpsimd.tensor_add(out=den[:], in0=a1[:], in1=p3[:])
                    rden = spool.tile([128, PW], fp32, tag="rden")
                    nc.vector.tensor_scalar_add(rden[:], den[:], EPS)
                    nc.vector.reciprocal_approx_fast(out=rden[:], in_=rden[:])

                    # gate g4 = 0.25*clip(1-clip(unc,0,1),0,1), zeroed on
                    # out-of-image pads (reproduces reference zero padding)
                    g4 = spool.tile([128, TILE_W], fp32, tag="g4")
                    nc.vector.tensor_scalar(
                        out=g4[:], in0=t_unc[:], scalar1=1.0, scalar2=0.0,
                        op0=Alu.min, op1=Alu.max)
                    nc.vector.tensor_scalar(
                        out=g4[:], in0=g4[:], scalar1=-0.25, scalar2=0.25,
                        op0=Alu.mult, op1=Alu.add)
                    if t_lo > W0:
                        nc.vector.memset(g4[:, W0:t_lo], 0.0)
                    if t_hi < W1:
                        nc.vector.memset(g4[:, t_hi:W1], 0.0)
                    if p_lo > 0:
                        nc.vector.memset(g4[0:p_lo, :], 0.0)
                    if p_hi < 128:
                        assert p_hi == 102
                        nc.vector.tensor_scalar(
                            out=g4[:], in0=g4[:], scalar1=botmask[:, 0:1],
                            scalar2=None, op0=Alu.mult)

                    g4r = spool.tile([128, PW], fp32, tag="g4r")
                    nc.gpsimd.tensor_mul(out=g4r[:], in0=g4[:, WSL], in1=rden[:])
                    Cxx = ppool.tile([128, PW], bf16, tag="Cxx")
                    Cyy = ppool.tile([128, PW], bf16, tag="Cyy")
                    Cxy = ppool.tile([128, PW], bf16, tag="Cxy")
                    G9 = ppool.tile([128, PW], bf16, tag="G9")
                    nc.vector.tensor_mul(out=Cxx[:], in0=g4r[:], in1=oxx[:])
                    nc.vector.tensor_mul(out=Cyy[:], in0=g4r[:], in1=oyy[:])
                    nc.gpsimd.tensor_mul(out=Cxy[:], in0=g4r[:], in1=oxy[:])
                    nc.scalar.mul(G9[:], g4[:, WSL], 1.0 / 9.0)

                    # ---- 6 iterations ----
                    s_cur = s0_b
                    for it in range(NUM_ITERS):
                        last = it == NUM_ITERS - 1
                        cs = spool.tile([128, TILE_W], bf16, tag="cs")
                        nc.vector.tensor_mul(out=cs[:], in0=cont_b[:],
                                             in1=s_cur[:])
                        hx = spool.tile([128, 526], bf16, tag="hx")
                        nc.vector.tensor_add(out=hx[:], in0=cs[:, 0:526],
                                             in1=cs[:, 2:528])
                        q1 = qpool.tile([128, PW], fp32, tag="q1")
                        q2 = qpool.tile([128, PW], fp32, tag="q2")
                        q3 = qpool.tile([128, PW], fp32, tag="q3")
                        q9 = qpool.tile([128, PW], fp32, tag="q9")
                        stencils(cs, hx, q1, q2, q3)
                        for lo in (0, 512):
                            hi = min(lo + 512, PW)
                            nc.tensor.matmul(q9[:, lo:hi], tT0I[:],
                                             s_cur[:, 1 + lo:1 + hi],
                                             start=True, stop=False)
                            nc.tensor.matmul(q9[:, lo:hi], tT0I[:],
                                             s_cur[:, 3 + lo:3 + hi],
                                             start=False, stop=False)
                            nc.tensor.matmul(q9[:, lo:hi], tT0I9[:],
                                             s_cur[:, 2 + lo:2 + hi],
                                             start=False, stop=True)

                        S1b = spool.tile([128, PW], bf16, tag="S1b")
                        S2b = spool.tile([128, PW], bf16, tag="S2b")
                        S3b = spool.tile([128, PW], bf16, tag="S3b")
                        Q9b = spool.tile([128, PW], bf16, tag="Q9b")
                        nc.scalar.copy(S1b[:], q1[:])
                        nc.scalar.copy(S2b[:], q2[:])
                        nc.scalar.copy(S3b[:], q3[:])
                        nc.scalar.copy(Q9b[:], q9[:])

                        m1 = spool.tile([128, PW], bf16, tag="m1")
                        m2 = spool.tile([128, PW], bf16, tag="m2")
                        m3 = spool.tile([128, PW], bf16, tag="m3")
                        n1 = spool.tile([128, PW], bf16, tag="n1")
                        num = spool.tile([128, PW], bf16, tag="num")
                        t2 = spool.tile([128, PW], bf16, tag="t2")
                        sd = spool.tile([128, PW], bf16, tag="sd")
                        nc.vector.tensor_mul(out=m1[:], in0=Cxx[:], in1=S1b[:])
                        nc.vector.tensor_mul(out=m2[:], in0=Cyy[:], in1=S2b[:])
                        nc.gpsimd.tensor_mul(out=m3[:], in0=Cxy[:], in1=S3b[:])
                        nc.vector.tensor_add(out=n1[:], in0=m1[:], in1=m2[:])
                        nc.vector.tensor_add(out=num[:], in0=n1[:], in1=m3[:])
                        nc.vector.tensor_mul(out=t2[:], in0=G9[:], in1=Q9b[:])
                        nc.gpsimd.tensor_add(out=sd[:], in0=t2[:], in1=num[:])
                        if last:
                            s_nxt = spool.tile([128, TILE_W], fp32, tag="s_f")
                        else:
                            s_nxt = s_ab[it % 2]
                        nc.vector.scalar_tensor_tensor(
                            out=s_nxt[:, WSL], in0=s_cur[:, WSL], scalar=0.75,
                            in1=sd[:], op0=Alu.mult, op1=Alu.add)
                        s_cur = s_nxt

                    nrows = r1 - r0
                    nc.sync.dma_start(
                        out=out_d[r0:r1, c0:c0 + 512],
                        in_=s_cur[6:6 + nrows, INT0:INT1])

    nc.finalize()
    return nc


def kernel(center, continuity, orientation, uncertainty):
    from concourse.bass_utils import run_bass_kernel_spmd

    if "nc" not in _CACHE:
        _CACHE["nc"] = _build_bass()
    nc = _CACHE["nc"]

    B = center.shape[0]
    in_maps = []
    for b in range(B):
        in_maps.append({
            "center": np.ascontiguousarray(center[b, 0]),
            "continuity": np.ascontiguousarray(continuity[b, 0]),
            "orientation": np.ascontiguousarray(orientation[b]),
            "uncertainty": np.ascontiguousarray(uncertainty[b, 0]),
        })
    res = run_bass_kernel_spmd(nc, in_maps, core_ids=list(range(B)))
    out = np.stack([r["out"] for r in res.results])[:, None]
    return out.astype(np.float32)


# revision 5
# speedup vs baseline: 1.9668x; 1.3155x over previous
"""Trainium2 Bass kernel for nn_IterativeStructureRefiner (v3, bf16, paired).

Math (validated vs reference: fp32 9e-8, bf16-quantized ~3e-3 l2rel):
  Rank-3 orientation factorization of the 8-neighbor affinity:
    num = oxx*S1 + oyy*S2 + oxy*S3,  S1 = Bm@hx, S2 = T0@cs + H0@hx,
    S3 = A0@csL - A0@csR   (cs = continuity*s, hx = csL+csR; vertical
    stencils are banded 128x128 stationaries on the TensorEngine).
  den = same stencils on continuity (iteration-invariant) -> folded with
  the uncertainty gate into precomputed coefficient maps:
    Cxx = 0.25*g*r*oxx (etc.), G9 = 0.25*g/9, r = 1/(den+eps).
  Smooth term in one PE pass: Q9 = T0I@sL + (T0I-9I)@sC + T0I@sR
    = 9*(box3x3(s) - s), so  s' = 0.75*s + G9*Q9 + Cxx*S1+Cyy*S2+Cxy*S3.
  The reference's final clip is provably inactive.

Scheduling: the two column-panel patches of a row panel are emitted
interleaved at iteration granularity, so each patch's serial dependency
chain (cs -> matmuls -> drains -> products -> update) fills the other's
engine gaps. PSUM rotates two tags x two buffers (8 banks total). All
loop tensors are bf16 with 4B-aligned windows (DVE 2x packing); odd-offset
shifted reads only feed the PE. PSUM->SBUF drains ride the Scalar engine,
GpSimd takes three elementwise ops per iteration.

Sharding: pure data-parallel, one batch image per NeuronCore (B=8).
9x2 patches of [128 rows x 524 cols] with 6px halo; all 6 iterations run
locally per patch -> inputs read once, output written once.
"""

import numpy as np

H = W = 1024
TILE_W = 528           # tile col t <-> image col c0 - 8 + t ; cols 0,527 pad
W0, W1 = 2, 526        # working window (524 cols)
PW = W1 - W0           # 524
INT0, INT1 = 8, 520    # interior output cols = image [c0, c0+512)
ROWS_OUT = 116
NUM_ITERS = 6
EPS = 1e-6

_CACHE = {}


def _build_bass():
    import concourse.bacc as bacc
    import concourse.mybir as mybir
    from concourse.tile import TileContext

    fp32 = mybir.dt.float32
    bf16 = mybir.dt.bfloat16
    Alu = mybir.AluOpType
    Act = mybir.ActivationFunctionType

    nc = bacc.Bacc("TRN2", debug=False)

    cen_d = nc.dram_tensor("center", [H, W], fp32, kind="ExternalInput")
    con_d = nc.dram_tensor("continuity", [H, W], fp32, kind="ExternalInput")
    ori_d = nc.dram_tensor("orientation", [2, H, W], fp32, kind="ExternalInput")
    unc_d = nc.dram_tensor("uncertainty", [H, W], fp32, kind="ExternalInput")
    out_d = nc.dram_tensor("out", [H, W], fp32, kind="ExternalOutput")

    k = np.arange(128)
    T0 = (np.abs(k[:, None] - k[None, :]) == 1).astype(np.float32)
    Bm = np.eye(128, dtype=np.float32) + 0.5 * T0
    H0 = 0.5 * T0
    A0 = ((k[:, None] == k[None, :] - 1).astype(np.float32)
          - (k[:, None] == k[None, :] + 1).astype(np.float32))
    A0m = -A0
    T0I = np.eye(128, dtype=np.float32) + T0
    T0I9 = T0 - 8.0 * np.eye(128, dtype=np.float32)
    ST = [Bm, T0, H0, A0, A0m, T0I, T0I9]
    st_drams = [nc.inline_tensor(m, name=f"st_{i}") for i, m in enumerate(ST)]
    botmask_np = (np.arange(128) < 102).astype(np.float32)[:, None]
    bot_dram = nc.inline_tensor(botmask_np, name="botmask")

    row_panels = []
    for r0 in range(0, H, ROWS_OUT):
        row_panels.append((r0, min(r0 + ROWS_OUT, H)))
    col_panels = [0, 512]

    with TileContext(nc) as tc:
        with (
            tc.tile_pool(name="consts", bufs=1) as cpool,
            tc.tile_pool(name="inp", bufs=2) as ipool,
            tc.tile_pool(name="pre", bufs=2) as ppool,
            tc.tile_pool(name="scr", bufs=2) as spool,
            tc.tile_pool(name="fpre", bufs=1) as fpool,
            tc.tile_pool(name="stg", bufs=1) as gpool,
            tc.tile_pool(name="outp", bufs=1) as opool,
            tc.tile_pool(name="psum", bufs=1, space="PSUM") as qpool,
        ):
            st_b = []
            for i, d in enumerate(st_drams):
                tf = gpool.tile([128, 128], fp32, tag=f"stf{i}")
                nc.sync.dma_start(out=tf[:], in_=d[:, :])
                tb = cpool.tile([128, 128], bf16, tag=f"st{i}")
                nc.scalar.copy(tb[:], tf[:])
                st_b.append(tb)
            tBm, tT0, tH0, tA0, tA0m, tT0I, tT0I9 = st_b
            botmask = cpool.tile([128, 1], fp32, tag="botmask")
            nc.sync.dma_start(out=botmask[:], in_=bot_dram[:, :])

            # per col-panel slot (A/B): persistent bf16 ping-pong s tiles
            s_ab = {}
            for sl in (0, 1):
                pair = []
                for nm in ("s_a", "s_b"):
                    t = cpool.tile([128, TILE_W], bf16, tag=f"{nm}{sl}")
                    nc.vector.memset(t[:, 0:W0], 0.0)
                    nc.vector.memset(t[:, W1:TILE_W], 0.0)
                    pair.append(t)
                s_ab[sl] = pair

            def stencils(src, hxt, q1t, q2t, q3t):
                """q1=Bm@hx, q2=T0@src+H0@hx, q3=A0@srcL-A0@srcR.
                Grouped per stationary to minimize LDWEIGHTS reloads."""
                CH = ((0, 512), (512, PW))
                for lo, hi in CH:
                    nc.tensor.matmul(q1t[:, lo:hi], tBm[:],
                                     hxt[:, 1 + lo:1 + hi],
                                     start=True, stop=True)
                for lo, hi in CH:
                    nc.tensor.matmul(q2t[:, lo:hi], tT0[:],
                                     src[:, 2 + lo:2 + hi],
                                     start=True, stop=False)
                for lo, hi in CH:
                    nc.tensor.matmul(q2t[:, lo:hi], tH0[:],
                                     hxt[:, 1 + lo:1 + hi],
                                     start=False, stop=True)
                for lo, hi in CH:
                    nc.tensor.matmul(q3t[:, lo:hi], tA0[:],
                                     src[:, 1 + lo:1 + hi],
                                     start=True, stop=False)
                for lo, hi in CH:
                    nc.tensor.matmul(q3t[:, lo:hi], tA0m[:],
                                     src[:, 3 + lo:3 + hi],
                                     start=False, stop=True)

            def qsmooth(s_cur, q9t):
                CH = ((0, 512), (512, PW))
                for off in (1, 3):
                    for lo, hi in CH:
                        nc.tensor.matmul(q9t[:, lo:hi], tT0I[:],
                                         s_cur[:, off + lo:off + hi],
                                         start=(off == 1), stop=False)
                for lo, hi in CH:
                    nc.tensor.matmul(q9t[:, lo:hi], tT0I9[:],
                                     s_cur[:, 2 + lo:2 + hi],
                                     start=False, stop=True)

            class Patch:
                pass

            def make_patch(r0, r1, c0, sl):
                P = Patch()
                P.r0, P.r1, P.c0, P.sl = r0, r1, c0, sl
                P.img_lo = max(c0 - 7, 0)
                P.img_hi = min(c0 + 519, W)
                P.t_lo = P.img_lo - (c0 - 8)
                P.t_hi = P.img_hi - (c0 - 8)
                P.row_lo = max(r0 - 6, 0)
                P.row_hi = min(r0 + 122, H)
                P.p_lo = P.row_lo - (r0 - 6)
                P.p_hi = P.row_hi - (r0 - 6)
                return P

            def emit_load(P):
                sl = P.sl

                def load(src_ap, tag):
                    t = ipool.tile([128, TILE_W], fp32, tag=f"{tag}{sl}")
                    if P.t_lo > 0:
                        nc.gpsimd.memset(t[:, 0:P.t_lo], 0.0)
                    if P.t_hi < TILE_W:
                        nc.gpsimd.memset(t[:, P.t_hi:TILE_W], 0.0)
                    if P.p_lo > 0:
                        nc.gpsimd.memset(t[0:P.p_lo, P.t_lo:P.t_hi], 0.0)
                    if P.p_hi < 128:
                        aligned_lo = (P.p_hi // 32) * 32
                        nc.gpsimd.memset(t[aligned_lo:128, P.t_lo:P.t_hi], 0.0)
                    nc.sync.dma_start(
                        out=t[P.p_lo:P.p_hi, P.t_lo:P.t_hi],
                        in_=src_ap[P.row_lo:P.row_hi, P.img_lo:P.img_hi])
                    return t

                P.t_cen = load(cen_d, "cen")
                P.t_con = load(con_d, "con")
                P.t_ox = load(ori_d[0], "ox")
                P.t_oy = load(ori_d[1], "oy")
                P.t_unc = load(unc_d, "unc")

            WSL = slice(W0, W1)

            def emit_pre(P):
                sl = P.sl
                P.cont_b = ppool.tile([128, TILE_W], bf16, tag=f"cont{sl}")
                P.s0_b = ppool.tile([128, TILE_W], bf16, tag=f"s0{sl}")
                nc.vector.tensor_scalar_mul(P.cont_b[:], P.t_con[:], 1.0)
                nc.vector.tensor_scalar_mul(P.s0_b[:], P.t_cen[:], 1.0)
                oxx = ppool.tile([128, PW], bf16, tag=f"oxx{sl}")
                oyy = ppool.tile([128, PW], bf16, tag=f"oyy{sl}")
                oxy = ppool.tile([128, PW], bf16, tag=f"oxy{sl}")
                nc.scalar.activation(oxx[:], P.t_ox[:, WSL], Act.Square)
                nc.scalar.activation(oyy[:], P.t_oy[:, WSL], Act.Square)
                nc.gpsimd.tensor_mul(out=oxy[:], in0=P.t_ox[:, WSL],
                                     in1=P.t_oy[:, WSL])

                hxC = spool.tile([128, 526], bf16, tag=f"hx{sl}")
                nc.vector.tensor_add(out=hxC[:], in0=P.cont_b[:, 0:526],
                                     in1=P.cont_b[:, 2:528])
                q1 = qpool.tile([128, PW], fp32, tag=f"qa{sl}")
                q2 = qpool.tile([128, PW], fp32, tag=f"qb{sl}")
                q3 = qpool.tile([128, PW], fp32, tag=f"qa{sl}")
                stencils(P.cont_b, hxC, q1, q2, q3)
                p1 = fpool.tile([128, PW], fp32, tag=f"p1{sl}")
                p2 = fpool.tile([128, PW], fp32, tag=f"p2{sl}")
                p3 = fpool.tile([128, PW], fp32, tag=f"p3{sl}")
                nc.vector.tensor_mul(out=p1[:], in0=oxx[:], in1=q1[:])
                nc.vector.tensor_mul(out=p2[:], in0=oyy[:], in1=q2[:])
                nc.vector.tensor_mul(out=p3[:], in0=oxy[:], in1=q3[:])
                a1 = fpool.tile([128, PW], fp32, tag=f"a1{sl}")
                den = fpool.tile([128, PW], fp32, tag=f"den{sl}")
                nc.gpsimd.tensor_add(out=a1[:], in0=p1[:], in1=p2[:])
                nc.gpsimd.tensor_add(out=den[:], in0=a1[:], in1=p3[:])
                rden = fpool.tile([128, PW], fp32, tag=f"rden{sl}")
                nc.vector.tensor_scalar_add(rden[:], den[:], EPS)
                nc.vector.reciprocal_approx_fast(out=rden[:], in_=rden[:])

                g4 = fpool.tile([128, TILE_W], fp32, tag=f"g4{sl}")
                nc.vector.tensor_scalar(
                    out=g4[:], in0=P.t_unc[:], scalar1=1.0, scalar2=0.0,
                    op0=Alu.min, op1=Alu.max)
                nc.vector.tensor_scalar(
                    out=g4[:], in0=g4[:], scalar1=-0.25, scalar2=0.25,
                    op0=Alu.mult, op1=Alu.add)
                if P.t_lo > W0:
                    nc.vector.memset(g4[:, W0:P.t_lo], 0.0)
                if P.t_hi < W1:
                    nc.vector.memset(g4[:, P.t_hi:W1], 0.0)
                if P.p_lo > 0:
                    nc.vector.memset(g4[0:P.p_lo, :], 0.0)
                if P.p_hi < 128:
                    assert P.p_hi == 102
                    nc.vector.tensor_scalar(
                        out=g4[:], in0=g4[:], scalar1=botmask[:, 0:1],
                        scalar2=None, op0=Alu.mult)

                g4r = fpool.tile([128, PW], fp32, tag=f"g4r{sl}")
                nc.gpsimd.tensor_mul(out=g4r[:], in0=g4[:, WSL], in1=rden[:])
                P.Cxx = ppool.tile([128, PW], bf16, tag=f"Cxx{sl}")
                P.Cyy = ppool.tile([128, PW], bf16, tag=f"Cyy{sl}")
                P.Cxy = ppool.tile([128, PW], bf16, tag=f"Cxy{sl}")
                P.G9 = ppool.tile([128, PW], bf16, tag=f"G9{sl}")
                nc.vector.tensor_mul(out=P.Cxx[:], in0=g4r[:], in1=oxx[:])
                nc.vector.tensor_mul(out=P.Cyy[:], in0=g4r[:], in1=oyy[:])
                nc.gpsimd.tensor_mul(out=P.Cxy[:], in0=g4r[:], in1=oxy[:])
                nc.scalar.mul(P.G9[:], g4[:, WSL], 1.0 / 9.0)
                P.s_cur = P.s0_b

            def emit_iter(P, it):
                sl = P.sl
                last = it == NUM_ITERS - 1
                s_cur = P.s_cur
                cs = spool.tile([128, TILE_W], bf16, tag=f"cs{sl}")
                nc.vector.tensor_mul(out=cs[:], in0=P.cont_b[:], in1=s_cur[:])
                hx = spool.tile([128, 526], bf16, tag=f"hx{sl}")
                nc.vector.tensor_add(out=hx[:], in0=cs[:, 0:526],
                                     in1=cs[:, 2:528])
                q1 = qpool.tile([128, PW], fp32, tag=f"qa{sl}")
                q2 = qpool.tile([128, PW], fp32, tag=f"qb{sl}")
                q3 = qpool.tile([128, PW], fp32, tag=f"qa{sl}")
                q9 = qpool.tile([128, PW], fp32, tag=f"qb{sl}")
                stencils(cs, hx, q1, q2, q3)
                qsmooth(s_cur, q9)

                S1b = spool.tile([128, PW], bf16, tag=f"S1b{sl}")
                S2b = spool.tile([128, PW], bf16, tag=f"S2b{sl}")
                S3b = spool.tile([128, PW], bf16, tag=f"S3b{sl}")
                Q9b = spool.tile([128, PW], bf16, tag=f"Q9b{sl}")
                nc.scalar.copy(S1b[:], q1[:])
                nc.scalar.copy(S2b[:], q2[:])
                nc.scalar.copy(S3b[:], q3[:])
                nc.scalar.copy(Q9b[:], q9[:])

                m1 = spool.tile([128, PW], bf16, tag=f"m1{sl}")
                m2 = spool.tile([128, PW], bf16, tag=f"m2{sl}")
                m3 = spool.tile([128, PW], bf16, tag=f"m3{sl}")
                n1 = spool.tile([128, PW], bf16, tag=f"n1{sl}")
                num = spool.tile([128, PW], bf16, tag=f"num{sl}")
                t2 = spool.tile([128, PW], bf16, tag=f"t2{sl}")
                sd = spool.tile([128, PW], bf16, tag=f"sd{sl}")
                nc.vector.tensor_mul(out=m1[:], in0=P.Cxx[:], in1=S1b[:])
                nc.vector.tensor_mul(out=m2[:], in0=P.Cyy[:], in1=S2b[:])
                nc.gpsimd.tensor_mul(out=m3[:], in0=P.Cxy[:], in1=S3b[:])
                nc.gpsimd.tensor_add(out=n1[:], in0=m1[:], in1=m2[:])
                nc.vector.tensor_mul(out=t2[:], in0=P.G9[:], in1=Q9b[:])
                nc.vector.tensor_add(out=num[:], in0=n1[:], in1=m3[:])
                nc.gpsimd.tensor_add(out=sd[:], in0=num[:], in1=t2[:])
                if last:
                    s075 = opool.tile([128, PW], fp32, tag=f"s075f{sl}")
                    s_nxt = opool.tile([128, TILE_W], fp32, tag=f"s_f{sl}")
                else:
                    s075 = spool.tile([128, PW], bf16, tag=f"s075{sl}")
                    s_nxt = s_ab[sl][it % 2]
                nc.vector.tensor_scalar_mul(s075[:], s_cur[:, WSL], 0.75)
                nc.vector.tensor_add(out=s_nxt[:, WSL], in0=s075[:], in1=sd[:])
                P.s_cur = s_nxt

            def emit_store(P):
                nrows = P.r1 - P.r0
                nc.sync.dma_start(
                    out=out_d[P.r0:P.r1, P.c0:P.c0 + 512],
                    in_=P.s_cur[6:6 + nrows, INT0:INT1])

            for (r0, r1) in row_panels:
                pa = make_patch(r0, r1, col_panels[0], 0)
                pb = make_patch(r0, r1, col_panels[1], 1)
                emit_load(pa)
                emit_load(pb)
                emit_pre(pa)
                emit_pre(pb)
                for it in range(NUM_ITERS):
                    emit_iter(pa, it)
                    emit_iter(pb, it)
                emit_store(pa)
                emit_store(pb)

    nc.finalize()
    return nc


def kernel(center, continuity, orientation, uncertainty):
    from concourse.bass_utils import run_bass_kernel_spmd

    if "nc" not in _CACHE:
        _CACHE["nc"] = _build_bass()
    nc = _CACHE["nc"]

    B = center.shape[0]
    in_maps = []
    for b in range(B):
        in_maps.append({
            "center": np.ascontiguousarray(center[b, 0]),
            "continuity": np.ascontiguousarray(continuity[b, 0]),
            "orientation": np.ascontiguousarray(orientation[b]),
            "uncertainty": np.ascontiguousarray(uncertainty[b, 0]),
        })
    res = run_bass_kernel_spmd(nc, in_maps, core_ids=list(range(B)))
    out = np.stack([r["out"] for r in res.results])[:, None]
    return out.astype(np.float32)
